# revision 1
# baseline (speedup 1.0000x reference)
"""NNUE HalfKP embedding-bag kernel for 8x Trainium2 NeuronCores.

Reference computation:
    stm_ft  = einsum('bk,bkf->bf', values, ft_w[stm_idx])  + ft_b
    nstm_ft = einsum('bk,bkf->bf', values, ft_w[nstm_idx]) + ft_b
    v_stm   = einsum('bk,bkf->bf', values, fft_w[stm_idx  % 640]) + fft_b
    v_nstm  = einsum('bk,bkf->bf', values, fft_w[nstm_idx % 640]) + fft_b
    hidden  = clip(concat([stm_ft + v_stm, nstm_ft + v_nstm]), 0, 1)   # [B, 1024]
    l1      = hidden @ out_w.T + out_b                                  # [B, 8]
    out     = sigmoid(l1[b, buckets[b]])                                # [B, 1]

Key restructurings:
  * ft_w[i] + fft_w[i % 640] share the same per-(b,k) weight, so gather from ONE
    combined table T[v] = ft_w[v] + fft_w[v % 640] in bf16 - 4x less gather
    traffic than two f32 table gathers.
  * Batch-shard across the 8 cores (1024 rows each); T replicated per core.
  * Gather via gpsimd.dma_gather (embedding gather: list position j -> SBUF
    partition j%128, slot j//128). Indices are int16, so the 40960-row table is
    split: per 128-row batch chunk the 4096 (b,k) lookups are compacted into a
    low-vocab list (v < 32768, <= 27 runs of 128) and a high-vocab list
    (v - 32768, <= 8 runs), padded with index 0 (weight 0).
  * The k-weighted sums run on the PE: per 128-slot run one [128,128]x[128,512]
    matmul whose host-prebuilt lhsT routes each slot p to its batch row m with
    weight values[b,k]; 35 runs accumulate in PSUM [128, 512].
  * Bucket selection folded host-side into w_sel = out_w[buckets] and
    out_b[buckets]; final logit = reduce(hidden * w_sel) + out_b_sel; sigmoid.
"""

import sys

sys.path.insert(0, "/opt/trn_rl_repo")

import numpy as np
import ml_dtypes

import concourse.bass as bass
import concourse.mybir as mybir
from concourse import bacc
from concourse.tile import TileContext
from concourse.bass_utils import run_bass_kernel_spmd

BF16 = ml_dtypes.bfloat16

B = 8192
K = 32
F = 512
FT_VOCAB = 40960
FFT_VOCAB = 640
N_CORES = 8
BC = B // N_CORES          # rows per core = 1024
CH = BC // 128             # chunks per core = 8
VSPLIT = 32768             # int16 index limit
LOW_RUNS = 27              # low-vocab runs of 128 slots (cap 3456 lookups)
HIGH_RUNS = 8              # high-vocab runs (cap 1024 lookups)
RUNS = LOW_RUNS + HIGH_RUNS
LOW_CAP = LOW_RUNS * 128
HIGH_CAP = HIGH_RUNS * 128
NIDX16 = (LOW_CAP + HIGH_CAP) // 16   # idx tensor cols (int16, 16-wrapped)

_compiled = None


def _build():
    nc = bacc.Bacc("TRN2", target_bir_lowering=False, debug=False, num_devices=N_CORES)

    t_d = nc.dram_tensor("t_tab", [FT_VOCAB + 1, F], mybir.dt.bfloat16, kind="ExternalInput")
    idx_d = {}
    w_d = {}
    for s in ("stm", "nstm"):
        idx_d[s] = nc.dram_tensor(f"idx_{s}", [CH, 128, NIDX16], mybir.dt.int16, kind="ExternalInput")
        w_d[s] = nc.dram_tensor(f"w_{s}", [CH, 128, RUNS * 128], mybir.dt.bfloat16, kind="ExternalInput")
    wsel_d = nc.dram_tensor("w_sel", [CH, 128, 2 * F], mybir.dt.bfloat16, kind="ExternalInput")
    obsel_d = nc.dram_tensor("ob_sel", [CH, 128, 1], mybir.dt.float32, kind="ExternalInput")
    out_d = nc.dram_tensor("out", [BC, 1], mybir.dt.float32, kind="ExternalOutput")

    with TileContext(nc) as tc:
        with (
            tc.tile_pool(name="const", bufs=1) as constp,
            tc.tile_pool(name="idx", bufs=4) as idxp,
            tc.tile_pool(name="gath", bufs=3) as gathp,
            tc.tile_pool(name="wblk", bufs=3) as wblkp,
            tc.tile_pool(name="psum", bufs=4, space="PSUM") as psump,
            tc.tile_pool(name="hid", bufs=2) as hidp,
            tc.tile_pool(name="wsel", bufs=2) as wselp,
            tc.tile_pool(name="fin", bufs=4) as finp,
        ):
            for ch in range(CH):
                hid = hidp.tile([128, 2 * F], mybir.dt.bfloat16)

                for s, sname in enumerate(("stm", "nstm")):
                    idxt = idxp.tile([128, NIDX16], mybir.dt.int16, tag="idx")
                    nc.sync.dma_start(out=idxt[:], in_=idx_d[sname][ch])

                    wblk = wblkp.tile([128, RUNS * 128], mybir.dt.bfloat16, tag="wblk")
                    nc.sync.dma_start(out=wblk[:], in_=w_d[sname][ch])

                    rt = gathp.tile([128, RUNS * F], mybir.dt.bfloat16, tag="gath")
                    nc.gpsimd.dma_gather(
                        out_ap=rt[:, : LOW_RUNS * F].rearrange("p (s f) -> p s f", f=F),
                        in_ap=t_d[:VSPLIT, :],
                        idxs_ap=idxt[:, : LOW_CAP // 16],
                        num_idxs=LOW_CAP,
                        num_idxs_reg=LOW_CAP,
                        elem_size=F,
                        single_packet=False,
                    )
                    nc.gpsimd.dma_gather(
                        out_ap=rt[:, LOW_RUNS * F :].rearrange("p (s f) -> p s f", f=F),
                        in_ap=t_d[VSPLIT:, :],
                        idxs_ap=idxt[:, LOW_CAP // 16 :],
                        num_idxs=HIGH_CAP,
                        num_idxs_reg=HIGH_CAP,
                        elem_size=F,
                        single_packet=False,
                    )

                    ps = psump.tile([128, F], mybir.dt.float32, tag="ps")
                    for q in range(RUNS):
                        nc.tensor.matmul(
                            out=ps[:],
                            lhsT=wblk[:, q * 128 : (q + 1) * 128],
                            rhs=rt[:, q * F : (q + 1) * F],
                            start=(q == 0),
                            stop=(q == RUNS - 1),
                        )

                    # hidden half = clip(psum, 0, 1) -> bf16 (bias rode a gather slot)
                    half = hid[:, s * F : (s + 1) * F]
                    nc.vector.tensor_scalar(
                        out=half,
                        in0=ps[:],
                        scalar1=0.0,
                        scalar2=1.0,
                        op0=mybir.AluOpType.max,
                        op1=mybir.AluOpType.min,
                    )

                wsel = wselp.tile([128, 2 * F], mybir.dt.bfloat16)
                nc.sync.dma_start(out=wsel[:], in_=wsel_d[ch])
                obsel = finp.tile([128, 1], mybir.dt.float32, tag="ob")
                nc.sync.dma_start(out=obsel[:], in_=obsel_d[ch])

                prod = finp.tile([128, 2 * F], mybir.dt.float32, tag="prod")
                nc.vector.tensor_tensor(
                    out=prod[:], in0=hid[:], in1=wsel[:], op=mybir.AluOpType.mult
                )
                acc = finp.tile([128, 1], mybir.dt.float32, tag="acc")
                nc.vector.tensor_reduce(
                    out=acc[:], in_=prod[:], axis=mybir.AxisListType.X, op=mybir.AluOpType.add
                )
                sig = finp.tile([128, 1], mybir.dt.float32, tag="sig")
                nc.scalar.activation(
                    out=sig[:],
                    in_=acc[:],
                    func=mybir.ActivationFunctionType.Sigmoid,
                    bias=obsel[:],
                )
                nc.sync.dma_start(out=out_d[ch * 128 : (ch + 1) * 128, :], in_=sig[:])

    nc.compile()
    return nc


def _get_compiled():
    global _compiled
    if _compiled is None:
        _compiled = _build()
    return _compiled


def _wrap16(lst):
    """int16 index list -> [128, len/16] wrapped (i -> [i%16, i//16]) + replicated."""
    n = lst.shape[0]
    w = lst.reshape(n // 16, 16).T.astype(np.int16)     # [16, n/16]
    return np.tile(w, (8, 1))                            # [128, n/16]


def _prep_set(idx_core, values_core):
    """Per-core index/weight prep for one index set.

    idx_core: [BC, K] int32, values_core: [BC, K] f32
    Returns idx16 [CH, 128, NIDX16] int16, W [CH, 128, RUNS*128] bf16.
    """
    idx16 = np.zeros((CH, 128, NIDX16), np.int16)
    W = np.zeros((CH, 128, RUNS, 128), np.float32)
    for ch in range(CH):
        v = idx_core[ch * 128 : (ch + 1) * 128].reshape(-1)       # [4096] entry j0 = m*K+k
        val = values_core[ch * 128 : (ch + 1) * 128].reshape(-1)
        m_of = np.repeat(np.arange(128), K)
        is_low = v < VSPLIT
        lc = int(is_low.sum())
        hc = 4096 - lc
        assert lc <= LOW_CAP and hc + 1 <= HIGH_CAP, (lc, hc)
        order = np.argsort(~is_low, kind="stable")
        lows, highs = order[:lc], order[lc:]

        ilist = np.zeros(LOW_CAP + HIGH_CAP, np.int16)
        ilist[:lc] = v[lows]
        ilist[LOW_CAP : LOW_CAP + hc] = v[highs] - VSPLIT
        ilist[LOW_CAP + hc] = FT_VOCAB - VSPLIT  # bias row of T, weight 1 for all m
        idx16[ch, :, : LOW_CAP // 16] = _wrap16(ilist[:LOW_CAP])
        idx16[ch, :, LOW_CAP // 16 :] = _wrap16(ilist[LOW_CAP:])

        pos = np.concatenate([np.arange(lc), LOW_CAP + np.arange(hc)])
        ent = np.concatenate([lows, highs])
        W[ch, pos % 128, pos // 128, m_of[ent]] = val[ent]
        bp = LOW_CAP + hc
        W[ch, bp % 128, bp // 128, :] = 1.0
    return idx16, np.ascontiguousarray(W.transpose(0, 1, 2, 3)).reshape(CH, 128, RUNS * 128).astype(BF16)


def _prep_core(core, T16, values, stm, nstm, wsel_all, obsel_all):
    rows = slice(core * BC, (core + 1) * BC)
    v_core = values[rows]
    idx_stm, w_stm = _prep_set(stm[rows], v_core)
    idx_nstm, w_nstm = _prep_set(nstm[rows], v_core)
    return {
        "t_tab": T16,
        "idx_stm": idx_stm,
        "idx_nstm": idx_nstm,
        "w_stm": w_stm,
        "w_nstm": w_nstm,
        "w_sel": wsel_all[rows].reshape(CH, 128, 2 * F).astype(BF16),
        "ob_sel": obsel_all[rows].reshape(CH, 128, 1).astype(np.float32),
    }


def build_in_maps(values, stm_indices, nstm_indices, buckets, ft_w, ft_b, fft_w, fft_b, out_w, out_b):
    values = np.asarray(values, dtype=np.float32)
    stm_indices = np.asarray(stm_indices, dtype=np.int32)
    nstm_indices = np.asarray(nstm_indices, dtype=np.int32)
    buckets = np.asarray(buckets, dtype=np.int32)
    ft_w = np.asarray(ft_w, dtype=np.float32)
    ft_b = np.asarray(ft_b, dtype=np.float32)
    fft_w = np.asarray(fft_w, dtype=np.float32)
    fft_b = np.asarray(fft_b, dtype=np.float32)
    out_w = np.asarray(out_w, dtype=np.float32)
    out_b = np.asarray(out_b, dtype=np.float32)

    T16 = np.concatenate([
        ft_w + np.tile(fft_w, (FT_VOCAB // FFT_VOCAB, 1)),
        (ft_b + fft_b).reshape(1, F),
    ]).astype(BF16)                               # [FT_VOCAB + 1, F]; last row = bias
    wsel_all = out_w[buckets]                     # [B, 1024] f32
    obsel_all = out_b[buckets]                    # [B] f32

    return [
        _prep_core(c, T16, values, stm_indices, nstm_indices, wsel_all, obsel_all)
        for c in range(N_CORES)
    ]


def kernel(**inputs):
    nc = _get_compiled()
    in_maps = build_in_maps(**inputs)
    res = run_bass_kernel_spmd(nc, in_maps, core_ids=list(range(N_CORES)))
    out = np.concatenate([res.results[c]["out"] for c in range(N_CORES)], axis=0)
    return out.astype(np.float32)

